# revision 23
# baseline (speedup 1.0000x reference)
"""Causal attention kernel for 8 TRN2 NeuronCores.

Problem: B=4, S=4096, D=1024 single-head causal attention with QKV projection.
  q/k/v = x @ W{q,k,v}.T ; out = softmax(tril(q k^T)/sqrt(D)) @ v

Sharding: core c -> batch b = c//2, parity p = c%2. Each core owns the 16 seq
blocks (128 rows) of batch b with block-index parity p ("striped" sequence
parallelism -> balanced causal work). There are NO collectives: each core
receives the full batch x (transposed and row-natural) from the host and
computes its own 2048 rows of output end to end.

Math restructuring vs the naive pipeline (all bf16 matmuls, f32 accum):
  scores = q k^T = x Wq^T Wk x^T = x M^T x^T with M = Wk^T Wq precomputed on
  the host, so no q/k projections exist on device at all; per 512-row q-group
  H = M x^T_group is built once ([1024, 512]) and scores come from
  s^T[k, q] = x^T . H. The softmax numerator P (=exp, unnormalized) is kept
  transposed [k, q]; V is never materialized either: U^T[d, q] = x^T-contract
  P over keys (lhsT = x rows natural), and ctx^T = Wv^T . U^T. The softmax
  denominator l comes from a ones-matmul (column sums, row-replicated)
  accumulated over key blocks, reciprocal'd once per group into a
  row-replicated [128, 512] tile that scales the final ctx^T eviction (so
  neither the U eviction nor the U-half handoff ever waits on it).

Causality is exact at 128-col granularity: for "band" key blocks the matmuls
are narrowed to the live q columns; the diagonal block gets a triangular
mask; one parity-dependent block column per other-parity band block is kept
or zeroed via a host-sent 0/1 mask (so the SPMD program is identical on all
cores and perfectly load-balanced).

PSUM (8 banks) is partitioned by tag: 2 rotating ("st": H/QK/C), 5 for the
U^T accumulator (built in two d-half passes over the key blocks; the 5th buf
lets the second half start while the first evicts), 1 for the l accumulator.
x^T and the 16 most-reused x-natural key blocks stay resident in SBUF; the
rest of x-natural streams per key block on the gpsimd DMA queue. DMA issue
occupies the issuing engine's instruction stream, so bulk loads live on the
sync/gpsimd queues (no compute) and the scalar queue only issues a few small
ones ahead of its exp/copy stream.
"""

import sys
import types

import numpy as np

sys.path.insert(0, "/opt/trn_rl_repo")

# run_bass_kernel_spmd imports antenv.axon_hooks when BASS_TRACE is set; if
# the module is absent in this environment, install a stub that reports "no
# hook" so tracing degrades gracefully instead of crashing the run.
try:
    import antenv.axon_hooks  # noqa: F401
except ImportError:
    _hook_mod = types.ModuleType("antenv.axon_hooks")
    _hook_mod._hook = None
    _hook_mod.set_axon_ntff_profile_hook = (
        lambda h: setattr(_hook_mod, "_hook", h)
    )
    _hook_mod.get_axon_ntff_profile_hook = lambda: _hook_mod._hook
    sys.modules["antenv.axon_hooks"] = _hook_mod

import concourse.bass as bass  # noqa: E402
import concourse.mybir as mybir  # noqa: E402
import concourse.tile as tile  # noqa: E402
from concourse import bacc  # noqa: E402
from concourse.bass_utils import run_bass_kernel_spmd  # noqa: E402

import ml_dtypes  # noqa: E402

B, S, D = 4, 4096, 1024
P = 128
NB = S // P          # 32 seq blocks per batch
NLB = NB // 2        # 16 own blocks per core
SH = S // 2          # 2048 own rows per core
NG = 4               # attention q-groups of 512 rows (4 local blocks each)
SCALE = 1.0 / 32.0   # 1/sqrt(D)

BF16 = mybir.dt.bfloat16
F32 = mybir.dt.float32

_built = {}


def _build_nc():
    nc = bacc.Bacc("TRN2", target_bir_lowering=False, debug=False, num_devices=8)

    # Host sends, per core (own-parity seq blocks FIRST, then other-parity):
    #   xtf:  x^T chunks [8, 128, 8*512] (chunk c = seq cols 512c..512c+511)
    #   xnf:  x row-natural per seq block [32, 128, 1024]
    #   mt:   (Wk^T Wq)^T in lhsT layout [128, 8, 1024]
    #   wvt:  Wv^T in lhsT layout [128, 2, 8, 512]
    #   masks: [:, :128] = lower-tri ones; [:, 128:] = parity mask (p ? 1 : 0)
    xtf = nc.declare_dram_parameter("xtf", [8, P, 8 * 512], BF16, isOutput=False)
    xnf = nc.declare_dram_parameter("xnf", [NB, P, D], BF16, isOutput=False)
    mt = nc.declare_dram_parameter("mt", [P, 8, D], BF16, isOutput=False)
    wvt = nc.declare_dram_parameter("wvt", [P, 2, 8, 512], BF16, isOutput=False)
    masks = nc.declare_dram_parameter("masks", [P, 2 * P], BF16, isOutput=False)
    y = nc.declare_dram_parameter("y", [D, SH], F32, isOutput=True)

    xtf3 = xtf.ap().rearrange("c p (po s) -> c p po s", po=8)   # [8, 128, 8, 512]
    xnf3 = xnf.ap()
    mt3 = mt.ap()
    wvt3 = wvt.ap()
    y3 = y.ap().rearrange("(ec pi) q -> ec pi q", pi=P)         # [8, 128, 2048]

    with tile.TileContext(nc) as tc:
        with (
            tc.tile_pool(name="consts", bufs=1) as consts,
            tc.tile_pool(name="mp", bufs=1) as mp,
            tc.tile_pool(name="wvp", bufs=1) as wvp,
            tc.tile_pool(name="xts", bufs=1) as xts,
            tc.tile_pool(name="xns", bufs=1) as xns,
            tc.tile_pool(name="hp", bufs=2) as hp,
            tc.tile_pool(name="strip", bufs=32) as strip,
            tc.tile_pool(name="vload", bufs=8) as vload,
            tc.tile_pool(name="linvp", bufs=2) as linvp,
            tc.tile_pool(name="unp", bufs=8) as unp,
            tc.tile_pool(name="ctxs", bufs=3) as ctxs,
            tc.tile_pool(name="psum", bufs=3, space="PSUM") as psum,
        ):
            masks_sb = consts.tile([P, 2 * P], BF16)
            ones_sb = consts.tile([P, P], BF16)
            nc.gpsimd.memset(ones_sb[:], 1.0)
            warm_sb = consts.tile([P, 512], BF16)
            nc.gpsimd.memset(warm_sb[:], 0.0)
            tri = masks_sb[:, 0:P]
            pmask = masks_sb[:, P:2 * P]

            mt_sb = mp.tile([P, 8, D], BF16)
            xt_sb = xts.tile([P, 8, S], BF16)        # x^T: [d, all 4096 rows]
            wv_sb = wvp.tile([P, 2, 8, 512], BF16)

            # Startup: H(0) eats one mt chunk + one x^T dc-slice per ~1.7us
            # burst, so mt is striped across all three DMA queues and x^T
            # chunk 0 is split per-dc; x^T chunk 4 (first other-parity keys,
            # needed by QK(0)) goes early on the gpsimd queue. Everything not
            # needed before ~45us (wv, resident-xn fills) sits at the back of
            # the sync/scalar queues so H(0)'s feed gets the HBM bandwidth.
            # Queue plan (DMA issue occupies the issuing engine's own
            # instruction stream, so bulk DMA goes on sync/gpsimd which run
            # no compute; scalar only issues 6 small ones before its exp/copy
            # stream starts):
            #   sync:   mt0, x^T own-half dc-slices (H(0)'s exact consumption
            #           order), resident-xn fills, then the y writes
            #   gpsimd: mt odd, x^T other-half dc-slices, then xn streams
            #   scalar: mt even, masks, wv
            nc.sync.dma_start(mt_sb[:, 0, 0:P], mt3[:, 0, 0:P])
            nc.sync.dma_start(mt_sb[:, 0, P:D], mt3[:, 0, P:D])
            for dcb in (1, 3):
                nc.gpsimd.dma_start(mt_sb[:, dcb], mt3[:, dcb])
            for dcb in (2, 4, 6):
                nc.scalar.dma_start(mt_sb[:, dcb], mt3[:, dcb])
            for dcb in range(8):
                nc.sync.dma_start(xt_sb[:, dcb, 0:512], xtf3[0][:, dcb, :])
            for dcb in (5, 7):
                nc.gpsimd.dma_start(mt_sb[:, dcb], mt3[:, dcb])
            nc.gpsimd.dma_start(xt_sb[:, :, 4 * 512:5 * 512], xtf3[4])
            # First 16 key-block slots of x-natural stay SBUF-resident (all of
            # groups 0-1's U reads, and the rect prefix of groups 2-3); only
            # slots >= 8 of each half are streamed per key block. All queues
            # are ordered by first-use time: xr slots 0-3/16-19 before the
            # x^T chunks that QK(1+) needs, the rest + wv on scalar.
            xr_sb = xns.tile([P, 16, D], BF16)

            def xr_fill(eng, slot):
                rix = slot if slot < 8 else 8 + (slot - NLB)
                eng.dma_start(xr_sb[:, rix, :], xnf3[slot])

            for slot in (0, 16, 1, 17):
                xr_fill(nc.sync, slot)
            for c in (1, 5):
                nc.sync.dma_start(xt_sb[:, :, c * 512:(c + 1) * 512], xtf3[c])
            for slot in (2, 18, 3, 19):
                xr_fill(nc.sync, slot)
            for c in (2, 6, 3, 7):
                nc.sync.dma_start(xt_sb[:, :, c * 512:(c + 1) * 512], xtf3[c])
            nc.scalar.dma_start(masks_sb[:], masks.ap())
            for slot in (4, 20, 5, 21, 6, 22, 7, 23):
                xr_fill(nc.scalar, slot)
            nc.scalar.dma_start(wv_sb[:, 0], wvt3[:, 0])
            nc.scalar.dma_start(wv_sb[:, 1], wvt3[:, 1])

            def w_ec(w_sb, dc, ec):
                return w_sb[:, ec // 4, dc, (ec % 4) * P:(ec % 4 + 1) * P]

            def emit_H(g, first=False):
                """H = M x^T for group g's own 512 rows -> h tile [128,8,512].
                g=0 runs dcb-outer across all 8 banks (mt chunks stream in
                while each dcb burst runs); later groups run db-outer with the
                rotating 3-bank ring so evictions trail progressively."""
                h_t = hp.tile([P, 8, 512], BF16, tag="h", name=f"h_{g}")
                rhs = xt_sb[:, :, g * 512:(g + 1) * 512]
                if first:
                    hts = (
                        [psum.tile([P, 512], F32, tag="u", bufs=5, name="h0u")
                         for _ in range(5)]
                        + [psum.tile([P, 512], F32, tag="st", bufs=2, name="h0s")
                           for _ in range(2)]
                        + [psum.tile([P, 512], F32, tag="lrep", bufs=1, name="h0l")]
                    )
                    for dcb in range(8):
                        for db in range(8):
                            nc.tensor.matmul(
                                hts[db][:],
                                lhsT=mt_sb[:, dcb, db * P:(db + 1) * P],
                                rhs=rhs[:, dcb, :],
                                start=(dcb == 0),
                                stop=(dcb == 7),
                            )
                    for db in range(8):
                        if db % 2 == 0:
                            nc.vector.tensor_copy(out=h_t[:, db, :], in_=hts[db][:])
                        else:
                            nc.scalar.copy(h_t[:, db, :], hts[db][:])
                else:
                    for db in range(8):
                        hps = psum.tile([P, 512], F32, tag="st", bufs=2,
                                        name=f"hps_{g}_{db}")
                        for dcb in range(8):
                            nc.tensor.matmul(
                                hps[:],
                                lhsT=mt_sb[:, dcb, db * P:(db + 1) * P],
                                rhs=rhs[:, dcb, :],
                                start=(dcb == 0),
                                stop=(dcb == 7),
                            )
                        if db % 2 == 0:
                            nc.vector.tensor_copy(out=h_t[:, db, :], in_=hps[:])
                        else:
                            nc.scalar.copy(h_t[:, db, :], hps[:])
                return h_t

            def emit_group(g, h_t):
                """QK + exp + mask + l, then U^T in two d-half passes, then
                H(g+1), then ctx^T = Wv^T Un^T and the y^T writeout."""
                nrect = 4 * g
                # (half, o): half 0 = own-parity keys, 1 = other-parity keys
                kbs = ([(0, o) for o in range(nrect)]
                       + [(1, o) for o in range(nrect)]
                       + [(0, nrect + r) for r in range(4)]
                       + [(1, nrect + r) for r in range(4)])
                nkb = len(kbs)

                def geom(half, o):
                    r = o - nrect
                    qoff = max(0, r) * P
                    return r, qoff, 512 - qoff

                lrep = psum.tile([P, 512], F32, tag="lrep", bufs=1,
                                 name=f"lrep_{g}")
                pts = []

                def l_mm(i):
                    half, o = kbs[i]
                    _, qoff, _ = geom(half, o)
                    nc.tensor.matmul(
                        lrep[:, qoff:512],
                        lhsT=ones_sb[:],
                        rhs=pts[i][:, qoff:512],
                        start=(i == 0),
                        stop=(i == nkb - 1),
                    )

                for idx, (half, o) in enumerate(kbs):
                    r, qoff, w = geom(half, o)
                    kcol = half * SH + o * P
                    st = psum.tile([P, 512], F32, tag="st", bufs=2,
                                   name=f"st_{g}")
                    for dc in range(8):
                        nc.tensor.matmul(
                            st[:, qoff:512],
                            lhsT=xt_sb[:, dc, kcol:kcol + P],
                            rhs=h_t[:, dc, qoff:512],
                            start=(dc == 0),
                            stop=(dc == 7),
                        )
                    pt = strip.tile([P, 512], BF16, tag="pt", name=f"pt_{g}")
                    nc.scalar.activation(
                        pt[:, qoff:512], st[:, qoff:512],
                        mybir.ActivationFunctionType.Exp, scale=SCALE,
                    )
                    if r >= 0:
                        m = tri if half == 0 else pmask
                        nc.vector.tensor_mul(
                            out=pt[:, qoff:qoff + P],
                            in0=pt[:, qoff:qoff + P], in1=m,
                        )
                    pts.append(pt)
                    if idx >= 1:
                        l_mm(idx - 1)
                # l_mm(nkb-1) is deferred into the U pass so the PE never
                # waits on the last key block's exp/mask.

                linv = linvp.tile([P, 512], F32, tag="linv", name=f"linv_{g}")
                un_list = [None] * 8
                for half_id in (0, 1):
                    u_ps = [psum.tile([P, 512], F32, tag="u", bufs=5,
                                      name=f"u_{g}_{half_id}_{i}")
                            for i in range(4)]
                    for idx, (half, o) in enumerate(kbs):
                        _, qoff, w = geom(half, o)
                        slot = o if half == 0 else NLB + o
                        if o < 8:
                            rix = o if half == 0 else 8 + o
                            xn_t = xr_sb[:, rix, half_id * 512:(half_id + 1) * 512]
                        else:
                            xn_t = vload.tile([P, 512], BF16, tag="xn",
                                              name=f"xn_{g}")
                            nc.gpsimd.dma_start(
                                xn_t[:],
                                xnf3[slot][:, half_id * 512:(half_id + 1) * 512],
                            )
                        for dcl in range(4):
                            nc.tensor.matmul(
                                u_ps[dcl][:, qoff:512],
                                lhsT=xn_t[:, dcl * P:(dcl + 1) * P],
                                rhs=pts[idx][:, qoff:512],
                                start=(idx == 0),
                                stop=(idx == nkb - 1),
                            )
                        if half_id == 0 and idx == 2:
                            l_mm(nkb - 1)
                            nc.vector.reciprocal(linv[:], lrep[:])
                    # Unnormalized eviction: 1/l is applied per-column at the
                    # C eviction instead, so nothing here waits on the
                    # reciprocal and U-B's bank reuse never stalls.
                    for dcl in range(4):
                        dc = half_id * 4 + dcl
                        un_t = unp.tile([P, 512], BF16, tag="un",
                                        name=f"un_{g}_{dc}")
                        if dcl % 2 == 0:
                            nc.vector.tensor_copy(out=un_t[:], in_=u_ps[dcl][:])
                        else:
                            nc.scalar.copy(un_t[:], u_ps[dcl][:])
                        un_list[dc] = un_t

                h_next = emit_H(g + 1) if g < NG - 1 else None

                for ec in range(8):
                    ctx_ps = psum.tile([P, 512], F32, tag="st", bufs=2,
                                       name=f"ctx_{g}_{ec}")
                    for dc in range(8):
                        nc.tensor.matmul(
                            ctx_ps[:],
                            lhsT=w_ec(wv_sb, dc, ec),
                            rhs=un_list[dc][:],
                            start=(dc == 0),
                            stop=(dc == 7),
                        )
                    cs = ctxs.tile([P, 512], F32, tag="cs", name=f"cs_{g}")
                    nc.vector.tensor_mul(out=cs[:], in0=ctx_ps[:], in1=linv[:])
                    nc.sync.dma_start(y3[ec][:, g * 512:(g + 1) * 512], cs[:])
                return h_next

            # Warm-up: ~3.5us of throwaway matmuls on memset data while the
            # first mt/x^T DMAs are still in flight -- opens the HAM clock
            # gate so the real stream starts at full rate instead of 1.2GHz.
            warm_ps = psum.tile([P, 512], F32, tag="st", bufs=2, name="warm_ps")
            for _ in range(14):
                nc.tensor.matmul(warm_ps[:], lhsT=ones_sb[:], rhs=warm_sb[:],
                                 start=True, stop=True)

            h_t = emit_H(0, first=True)
            for g in range(NG):
                h_t = emit_group(g, h_t)

    nc.compile()
    return nc


def _host_inputs(x, Wq, Wk, Wv):
    """Build per-core input maps. x: [B,S,D] f32; W*: [D,D] f32."""
    bf = ml_dtypes.bfloat16

    # Merged score weight: scores = q k^T = x M^T x^T, M = Wk^T Wq.
    # lhsT layout for H = M x^T: mt[pi, po, a] = M[a, po*128+pi].
    M = Wk.T.astype(np.float32) @ Wq.astype(np.float32)
    mt = np.ascontiguousarray(
        M.T.reshape(8, P, D).transpose(1, 0, 2)
    ).astype(bf)

    def w_pim(W):
        # [pi, eh, po, e'] with element = W[eh*512+e', po*128+pi]
        return np.ascontiguousarray(
            W.T.astype(bf).reshape(8, P, 2, 512).transpose(1, 2, 0, 3)
        )

    wvt = w_pim(Wv)

    kj = np.arange(P)[:, None]
    qr = np.arange(P)[None, :]
    tri = (kj <= qr).astype(np.float32)

    in_maps = []
    cache = {}
    for c in range(8):
        b, p = c // 2, c % 2
        if (b, p) not in cache:
            # own-parity seq blocks first, then the other parity
            perm = ([2 * j + p for j in range(NLB)]
                    + [2 * j + (1 - p) for j in range(NLB)])
            xbf = x[b].reshape(NB, P, D)[perm].reshape(S, D)
            xt_full = xbf.T.astype(bf)  # [D, S]
            xtf_c = np.ascontiguousarray(
                xt_full.reshape(8, P, 8, 512).transpose(2, 1, 0, 3)
            ).reshape(8, P, 8 * 512)
            xnf_c = np.ascontiguousarray(xbf.astype(bf).reshape(NB, P, D))
            cache[(b, p)] = (xtf_c, xnf_c)
        xtf_c, xnf_c = cache[(b, p)]
        pm = np.full((P, P), 1.0 if p == 1 else 0.0, np.float32)
        in_maps.append({
            "xtf": xtf_c,
            "xnf": xnf_c,
            "mt": mt,
            "wvt": wvt,
            "masks": np.concatenate([tri, pm], axis=1).astype(bf),
        })
    return in_maps


def kernel(**inputs):
    x = np.asarray(inputs["inputs"], np.float32)
    Wq = np.asarray(inputs["Wq"], np.float32)
    Wk = np.asarray(inputs["Wk"], np.float32)
    Wv = np.asarray(inputs["Wv"], np.float32)

    if "nc" not in _built:
        _built["nc"] = _build_nc()
    nc = _built["nc"]

    in_maps = _host_inputs(x, Wq, Wk, Wv)
    res = run_bass_kernel_spmd(nc, in_maps, core_ids=list(range(8)))

    out = np.empty((B, S, D), np.float32)
    for c in range(8):
        b, p = c // 2, c % 2
        yc = res.results[c]["y"]  # [1024, 2048] = ctx^T, own rows slot-major
        ob = out[b].reshape(NB, P, D)
        for j in range(NLB):
            ob[2 * j + p] = yc[:, j * P:(j + 1) * P].T
    return out


# revision 24
# speedup vs baseline: 1.0043x; 1.0043x over previous
"""Causal attention kernel for 8 TRN2 NeuronCores.

Problem: B=4, S=4096, D=1024 single-head causal attention with QKV projection.
  q/k/v = x @ W{q,k,v}.T ; out = softmax(tril(q k^T)/sqrt(D)) @ v

Sharding: core c -> batch b = c//2, parity p = c%2. Each core owns the 16 seq
blocks (128 rows) of batch b with block-index parity p ("striped" sequence
parallelism -> balanced causal work). There are NO collectives: each core
receives the full batch x (transposed and row-natural) from the host and
computes its own 2048 rows of output end to end.

Math restructuring vs the naive pipeline (all bf16 matmuls, f32 accum):
  scores = q k^T = x Wq^T Wk x^T = x M^T x^T with M = Wk^T Wq precomputed on
  the host, so no q/k projections exist on device at all; per 512-row q-group
  H = M x^T_group is built once ([1024, 512]) and scores come from
  s^T[k, q] = x^T . H. The softmax numerator P (=exp, unnormalized) is kept
  transposed [k, q]; V is never materialized either: U^T[d, q] = x^T-contract
  P over keys (lhsT = x rows natural), and ctx^T = Wv^T . U^T. The softmax
  denominator l comes from a ones-matmul (column sums, row-replicated)
  accumulated over key blocks, reciprocal'd once per group into a
  row-replicated [128, 512] tile that scales the final ctx^T eviction (so
  neither the U eviction nor the U-half handoff ever waits on it).

Causality is exact at 128-col granularity: for "band" key blocks the matmuls
are narrowed to the live q columns; the diagonal block gets a triangular
mask; one parity-dependent block column per other-parity band block is kept
or zeroed via a host-sent 0/1 mask (so the SPMD program is identical on all
cores and perfectly load-balanced).

PSUM (8 banks) is partitioned by tag: 2 rotating ("st": H/QK/C), 5 for the
U^T accumulator (built in two d-half passes over the key blocks; the 5th buf
lets the second half start while the first evicts), 1 for the l accumulator.
x^T and the 16 most-reused x-natural key blocks stay resident in SBUF; the
rest of x-natural streams per key block on the gpsimd DMA queue. DMA issue
occupies the issuing engine's instruction stream, so bulk loads live on the
sync/gpsimd queues (no compute) and the scalar queue only issues a few small
ones ahead of its exp/copy stream.
"""

import sys
import types

import numpy as np

sys.path.insert(0, "/opt/trn_rl_repo")

# run_bass_kernel_spmd imports antenv.axon_hooks when BASS_TRACE is set; if
# the module is absent in this environment, install a stub that reports "no
# hook" so tracing degrades gracefully instead of crashing the run.
try:
    import antenv.axon_hooks  # noqa: F401
except ImportError:
    _hook_mod = types.ModuleType("antenv.axon_hooks")
    _hook_mod._hook = None
    _hook_mod.set_axon_ntff_profile_hook = (
        lambda h: setattr(_hook_mod, "_hook", h)
    )
    _hook_mod.get_axon_ntff_profile_hook = lambda: _hook_mod._hook
    sys.modules["antenv.axon_hooks"] = _hook_mod

import concourse.bass as bass  # noqa: E402
import concourse.mybir as mybir  # noqa: E402
import concourse.tile as tile  # noqa: E402
from concourse import bacc  # noqa: E402
from concourse.bass_utils import run_bass_kernel_spmd  # noqa: E402

import ml_dtypes  # noqa: E402

B, S, D = 4, 4096, 1024
P = 128
NB = S // P          # 32 seq blocks per batch
NLB = NB // 2        # 16 own blocks per core
SH = S // 2          # 2048 own rows per core
NG = 4               # attention q-groups of 512 rows (4 local blocks each)
SCALE = 1.0 / 32.0   # 1/sqrt(D)

BF16 = mybir.dt.bfloat16
F32 = mybir.dt.float32

_built = {}


def _build_nc():
    nc = bacc.Bacc("TRN2", target_bir_lowering=False, debug=False, num_devices=8)

    # Host sends, per core (own-parity seq blocks FIRST, then other-parity):
    #   xtf:  x^T chunks [8, 128, 8*512] (chunk c = seq cols 512c..512c+511)
    #   xnf:  x row-natural per seq block [32, 128, 1024]
    #   mt:   (Wk^T Wq)^T in lhsT layout [128, 8, 1024]
    #   wvt:  Wv^T in lhsT layout [128, 2, 8, 512]
    #   masks: [:, :128] = lower-tri ones; [:, 128:] = parity mask (p ? 1 : 0)
    xtf = nc.declare_dram_parameter("xtf", [8, P, 8 * 512], BF16, isOutput=False)
    xnf = nc.declare_dram_parameter("xnf", [NB, P, D], BF16, isOutput=False)
    mt = nc.declare_dram_parameter("mt", [P, 8, D], BF16, isOutput=False)
    wvt = nc.declare_dram_parameter("wvt", [P, 2, 8, 512], BF16, isOutput=False)
    masks = nc.declare_dram_parameter("masks", [P, 2 * P], BF16, isOutput=False)
    y = nc.declare_dram_parameter("y", [D, SH], F32, isOutput=True)

    xtf3 = xtf.ap().rearrange("c p (po s) -> c p po s", po=8)   # [8, 128, 8, 512]
    xnf3 = xnf.ap()
    mt3 = mt.ap()
    wvt3 = wvt.ap()
    y3 = y.ap().rearrange("(ec pi) q -> ec pi q", pi=P)         # [8, 128, 2048]

    with tile.TileContext(nc) as tc:
        with (
            tc.tile_pool(name="consts", bufs=1) as consts,
            tc.tile_pool(name="mp", bufs=1) as mp,
            tc.tile_pool(name="wvp", bufs=1) as wvp,
            tc.tile_pool(name="xts", bufs=1) as xts,
            tc.tile_pool(name="xns", bufs=1) as xns,
            tc.tile_pool(name="hp", bufs=2) as hp,
            tc.tile_pool(name="strip", bufs=32) as strip,
            tc.tile_pool(name="vload", bufs=8) as vload,
            tc.tile_pool(name="linvp", bufs=2) as linvp,
            tc.tile_pool(name="unp", bufs=8) as unp,
            tc.tile_pool(name="ctxs", bufs=3) as ctxs,
            tc.tile_pool(name="psum", bufs=3, space="PSUM") as psum,
        ):
            masks_sb = consts.tile([P, 2 * P], BF16)
            ones_sb = consts.tile([P, P], BF16)
            nc.gpsimd.memset(ones_sb[:], 1.0)
            warm_sb = consts.tile([P, 512], BF16)
            nc.gpsimd.memset(warm_sb[:], 0.0)
            tri = masks_sb[:, 0:P]
            pmask = masks_sb[:, P:2 * P]

            mt_sb = mp.tile([P, 8, D], BF16)
            xt_sb = xts.tile([P, 8, S], BF16)        # x^T: [d, all 4096 rows]
            wv_sb = wvp.tile([P, 2, 8, 512], BF16)

            # Startup: H(0) eats one mt chunk + one x^T dc-slice per ~1.7us
            # burst, so mt is striped across all three DMA queues and x^T
            # chunk 0 is split per-dc; x^T chunk 4 (first other-parity keys,
            # needed by QK(0)) goes early on the gpsimd queue. Everything not
            # needed before ~45us (wv, resident-xn fills) sits at the back of
            # the sync/scalar queues so H(0)'s feed gets the HBM bandwidth.
            # Queue plan (DMA issue occupies the issuing engine's own
            # instruction stream, so bulk DMA goes on sync/gpsimd which run
            # no compute; scalar only issues 6 small ones before its exp/copy
            # stream starts):
            #   sync:   mt0, x^T own-half dc-slices (H(0)'s exact consumption
            #           order), resident-xn fills, then the y writes
            #   gpsimd: mt odd, x^T other-half dc-slices, then xn streams
            #   scalar: mt even, masks, wv
            nc.sync.dma_start(mt_sb[:, 0], mt3[:, 0])
            for dcb in (1, 3):
                nc.gpsimd.dma_start(mt_sb[:, dcb], mt3[:, dcb])
            for dcb in (2, 4, 6):
                nc.scalar.dma_start(mt_sb[:, dcb], mt3[:, dcb])
            for dcb in range(8):
                nc.sync.dma_start(xt_sb[:, dcb, 0:512], xtf3[0][:, dcb, :])
            for dcb in (5, 7):
                nc.gpsimd.dma_start(mt_sb[:, dcb], mt3[:, dcb])
            nc.gpsimd.dma_start(xt_sb[:, :, 4 * 512:5 * 512], xtf3[4])
            # First 16 key-block slots of x-natural stay SBUF-resident (all of
            # groups 0-1's U reads, and the rect prefix of groups 2-3); only
            # slots >= 8 of each half are streamed per key block. All queues
            # are ordered by first-use time: xr slots 0-3/16-19 before the
            # x^T chunks that QK(1+) needs, the rest + wv on scalar.
            xr_sb = xns.tile([P, 16, D], BF16)

            def xr_fill(eng, slot):
                rix = slot if slot < 8 else 8 + (slot - NLB)
                eng.dma_start(xr_sb[:, rix, :], xnf3[slot])

            for slot in (0, 16, 1, 17):
                xr_fill(nc.sync, slot)
            for c in (1, 5):
                nc.sync.dma_start(xt_sb[:, :, c * 512:(c + 1) * 512], xtf3[c])
            for slot in (2, 18, 3, 19):
                xr_fill(nc.sync, slot)
            for c in (2, 6, 3, 7):
                nc.sync.dma_start(xt_sb[:, :, c * 512:(c + 1) * 512], xtf3[c])
            nc.scalar.dma_start(masks_sb[:], masks.ap())
            for slot in (4, 20, 5, 21, 6, 22, 7, 23):
                xr_fill(nc.scalar, slot)
            nc.scalar.dma_start(wv_sb[:, 0], wvt3[:, 0])
            nc.scalar.dma_start(wv_sb[:, 1], wvt3[:, 1])

            def w_ec(w_sb, dc, ec):
                return w_sb[:, ec // 4, dc, (ec % 4) * P:(ec % 4 + 1) * P]

            def emit_H(g, first=False):
                """H = M x^T for group g's own 512 rows -> h tile [128,8,512].
                g=0 runs dcb-outer across all 8 banks (mt chunks stream in
                while each dcb burst runs); later groups run db-outer with the
                rotating 3-bank ring so evictions trail progressively."""
                h_t = hp.tile([P, 8, 512], BF16, tag="h", name=f"h_{g}")
                rhs = xt_sb[:, :, g * 512:(g + 1) * 512]
                if first:
                    hts = (
                        [psum.tile([P, 512], F32, tag="u", bufs=5, name="h0u")
                         for _ in range(5)]
                        + [psum.tile([P, 512], F32, tag="st", bufs=2, name="h0s")
                           for _ in range(2)]
                        + [psum.tile([P, 512], F32, tag="lrep", bufs=1, name="h0l")]
                    )
                    for dcb in range(8):
                        for db in range(8):
                            nc.tensor.matmul(
                                hts[db][:],
                                lhsT=mt_sb[:, dcb, db * P:(db + 1) * P],
                                rhs=rhs[:, dcb, :],
                                start=(dcb == 0),
                                stop=(dcb == 7),
                            )
                    for db in range(8):
                        if db % 2 == 0:
                            nc.vector.tensor_copy(out=h_t[:, db, :], in_=hts[db][:])
                        else:
                            nc.scalar.copy(h_t[:, db, :], hts[db][:])
                else:
                    for db in range(8):
                        hps = psum.tile([P, 512], F32, tag="st", bufs=2,
                                        name=f"hps_{g}_{db}")
                        for dcb in range(8):
                            nc.tensor.matmul(
                                hps[:],
                                lhsT=mt_sb[:, dcb, db * P:(db + 1) * P],
                                rhs=rhs[:, dcb, :],
                                start=(dcb == 0),
                                stop=(dcb == 7),
                            )
                        if db % 2 == 0:
                            nc.vector.tensor_copy(out=h_t[:, db, :], in_=hps[:])
                        else:
                            nc.scalar.copy(h_t[:, db, :], hps[:])
                return h_t

            def emit_group(g, h_t):
                """QK + exp + mask + l, then U^T in two d-half passes, then
                H(g+1), then ctx^T = Wv^T Un^T and the y^T writeout."""
                nrect = 4 * g
                # (half, o): half 0 = own-parity keys, 1 = other-parity keys
                kbs = ([(0, o) for o in range(nrect)]
                       + [(1, o) for o in range(nrect)]
                       + [(0, nrect + r) for r in range(4)]
                       + [(1, nrect + r) for r in range(4)])
                nkb = len(kbs)

                def geom(half, o):
                    r = o - nrect
                    qoff = max(0, r) * P
                    return r, qoff, 512 - qoff

                lrep = psum.tile([P, 512], F32, tag="lrep", bufs=1,
                                 name=f"lrep_{g}")
                pts = []

                def l_mm(i):
                    half, o = kbs[i]
                    _, qoff, _ = geom(half, o)
                    nc.tensor.matmul(
                        lrep[:, qoff:512],
                        lhsT=ones_sb[:],
                        rhs=pts[i][:, qoff:512],
                        start=(i == 0),
                        stop=(i == nkb - 1),
                    )

                for idx, (half, o) in enumerate(kbs):
                    r, qoff, w = geom(half, o)
                    kcol = half * SH + o * P
                    st = psum.tile([P, 512], F32, tag="st", bufs=2,
                                   name=f"st_{g}")
                    for dc in range(8):
                        nc.tensor.matmul(
                            st[:, qoff:512],
                            lhsT=xt_sb[:, dc, kcol:kcol + P],
                            rhs=h_t[:, dc, qoff:512],
                            start=(dc == 0),
                            stop=(dc == 7),
                        )
                    pt = strip.tile([P, 512], BF16, tag="pt", name=f"pt_{g}")
                    nc.scalar.activation(
                        pt[:, qoff:512], st[:, qoff:512],
                        mybir.ActivationFunctionType.Exp, scale=SCALE,
                    )
                    if r >= 0:
                        m = tri if half == 0 else pmask
                        nc.vector.tensor_mul(
                            out=pt[:, qoff:qoff + P],
                            in0=pt[:, qoff:qoff + P], in1=m,
                        )
                    pts.append(pt)
                    if idx >= 1:
                        l_mm(idx - 1)
                # l_mm(nkb-1) is deferred into the U pass so the PE never
                # waits on the last key block's exp/mask.

                linv = linvp.tile([P, 512], F32, tag="linv", name=f"linv_{g}")
                un_list = [None] * 8
                for half_id in (0, 1):
                    u_ps = [psum.tile([P, 512], F32, tag="u", bufs=5,
                                      name=f"u_{g}_{half_id}_{i}")
                            for i in range(4)]
                    for idx, (half, o) in enumerate(kbs):
                        _, qoff, w = geom(half, o)
                        slot = o if half == 0 else NLB + o
                        if o < 8:
                            rix = o if half == 0 else 8 + o
                            xn_t = xr_sb[:, rix, half_id * 512:(half_id + 1) * 512]
                        else:
                            xn_t = vload.tile([P, 512], BF16, tag="xn",
                                              name=f"xn_{g}")
                            nc.gpsimd.dma_start(
                                xn_t[:],
                                xnf3[slot][:, half_id * 512:(half_id + 1) * 512],
                            )
                        for dcl in range(4):
                            nc.tensor.matmul(
                                u_ps[dcl][:, qoff:512],
                                lhsT=xn_t[:, dcl * P:(dcl + 1) * P],
                                rhs=pts[idx][:, qoff:512],
                                start=(idx == 0),
                                stop=(idx == nkb - 1),
                            )
                        if half_id == 0 and idx == 2:
                            l_mm(nkb - 1)
                            nc.vector.reciprocal(linv[:], lrep[:])
                    # Unnormalized eviction: 1/l is applied per-column at the
                    # C eviction instead, so nothing here waits on the
                    # reciprocal and U-B's bank reuse never stalls.
                    for dcl in range(4):
                        dc = half_id * 4 + dcl
                        un_t = unp.tile([P, 512], BF16, tag="un",
                                        name=f"un_{g}_{dc}")
                        if dcl % 2 == 0:
                            nc.vector.tensor_copy(out=un_t[:], in_=u_ps[dcl][:])
                        else:
                            nc.scalar.copy(un_t[:], u_ps[dcl][:])
                        un_list[dc] = un_t

                h_next = emit_H(g + 1) if g < NG - 1 else None

                for ec in range(8):
                    ctx_ps = psum.tile([P, 512], F32, tag="st", bufs=2,
                                       name=f"ctx_{g}_{ec}")
                    for dc in range(8):
                        nc.tensor.matmul(
                            ctx_ps[:],
                            lhsT=w_ec(wv_sb, dc, ec),
                            rhs=un_list[dc][:],
                            start=(dc == 0),
                            stop=(dc == 7),
                        )
                    cs = ctxs.tile([P, 512], F32, tag="cs", name=f"cs_{g}")
                    nc.vector.tensor_mul(out=cs[:], in0=ctx_ps[:], in1=linv[:])
                    nc.sync.dma_start(y3[ec][:, g * 512:(g + 1) * 512], cs[:])
                return h_next

            # Warm-up: ~3.5us of throwaway matmuls on memset data while the
            # first mt/x^T DMAs are still in flight -- opens the HAM clock
            # gate so the real stream starts at full rate instead of 1.2GHz.
            warm_ps = psum.tile([P, 512], F32, tag="st", bufs=2, name="warm_ps")
            for _ in range(22):
                nc.tensor.matmul(warm_ps[:], lhsT=ones_sb[:], rhs=warm_sb[:],
                                 start=True, stop=True)

            h_t = emit_H(0, first=True)
            for g in range(NG):
                h_t = emit_group(g, h_t)

    nc.compile()
    return nc


def _host_inputs(x, Wq, Wk, Wv):
    """Build per-core input maps. x: [B,S,D] f32; W*: [D,D] f32."""
    bf = ml_dtypes.bfloat16

    # Merged score weight: scores = q k^T = x M^T x^T, M = Wk^T Wq.
    # lhsT layout for H = M x^T: mt[pi, po, a] = M[a, po*128+pi].
    M = Wk.T.astype(np.float32) @ Wq.astype(np.float32)
    mt = np.ascontiguousarray(
        M.T.reshape(8, P, D).transpose(1, 0, 2)
    ).astype(bf)

    def w_pim(W):
        # [pi, eh, po, e'] with element = W[eh*512+e', po*128+pi]
        return np.ascontiguousarray(
            W.T.astype(bf).reshape(8, P, 2, 512).transpose(1, 2, 0, 3)
        )

    wvt = w_pim(Wv)

    kj = np.arange(P)[:, None]
    qr = np.arange(P)[None, :]
    tri = (kj <= qr).astype(np.float32)

    in_maps = []
    cache = {}
    for c in range(8):
        b, p = c // 2, c % 2
        if (b, p) not in cache:
            # own-parity seq blocks first, then the other parity
            perm = ([2 * j + p for j in range(NLB)]
                    + [2 * j + (1 - p) for j in range(NLB)])
            xbf = x[b].reshape(NB, P, D)[perm].reshape(S, D)
            xt_full = xbf.T.astype(bf)  # [D, S]
            xtf_c = np.ascontiguousarray(
                xt_full.reshape(8, P, 8, 512).transpose(2, 1, 0, 3)
            ).reshape(8, P, 8 * 512)
            xnf_c = np.ascontiguousarray(xbf.astype(bf).reshape(NB, P, D))
            cache[(b, p)] = (xtf_c, xnf_c)
        xtf_c, xnf_c = cache[(b, p)]
        pm = np.full((P, P), 1.0 if p == 1 else 0.0, np.float32)
        in_maps.append({
            "xtf": xtf_c,
            "xnf": xnf_c,
            "mt": mt,
            "wvt": wvt,
            "masks": np.concatenate([tri, pm], axis=1).astype(bf),
        })
    return in_maps


def kernel(**inputs):
    x = np.asarray(inputs["inputs"], np.float32)
    Wq = np.asarray(inputs["Wq"], np.float32)
    Wk = np.asarray(inputs["Wk"], np.float32)
    Wv = np.asarray(inputs["Wv"], np.float32)

    if "nc" not in _built:
        _built["nc"] = _build_nc()
    nc = _built["nc"]

    in_maps = _host_inputs(x, Wq, Wk, Wv)
    res = run_bass_kernel_spmd(nc, in_maps, core_ids=list(range(8)))

    out = np.empty((B, S, D), np.float32)
    for c in range(8):
        b, p = c // 2, c % 2
        yc = res.results[c]["y"]  # [1024, 2048] = ctx^T, own rows slot-major
        ob = out[b].reshape(NB, P, D)
        for j in range(NLB):
            ob[2 * j + p] = yc[:, j * P:(j + 1) * P].T
    return out
